# revision 2
# baseline (speedup 1.0000x reference)
"""Equivariant PQ-layer conv kernel for 8x TRN2 NeuronCores — v3.

Sharding: 125 taps = 25 (dy,dx) columns x 5 dz. Cores own 3 columns each
(columns 0..23); column 24 is split across cores as 2 row-chunks each.
Within a column, the 5 taps differ only by a z-shift, so x ships as 5
per-core (dy,dx)-translated window tiles (128ch, 12z, 8y, 8x) that the PE
slices with program-constant 3D APs — no im2col duplication. Tap pairs
(dz,dz+1) chain the 320-row contraction into full 128-row K-chunks; the
chain's mixed chunk uses a tile whose upper 64 partitions hold ch 0:64
pre-shifted one z-slice. Transposed matmuls: psum[m] (128 vox, 320 A)
accumulates all 41 K-chunks; out partials are fp16, summed on host.
"""
import numpy as np

C0, C1 = 8, 4
K = 5
G = 8
EPS = 1e-6
R_MAX = 5.5
DIM = C0 + 3 * C1          # 20
Q = 16
P = 8
NCH = DIM * Q              # 320
NV = P * P * P             # 512
K3 = K ** 5 // K ** 2      # 125
NCORES = 8
NSLOT = 3                  # columns per core
NCPC = 13                  # K-chunks per column
NEX = 2                    # extra chunks (column 24 split)
NCHUNK = NSLOT * NCPC + NEX  # 41
NM = 4                     # vox M-chunks: 512/128
WARMUP = 72

LAST = None
_PROGRAM = None


def _levi_civita():
    e = np.zeros((3, 3, 3), np.float32)
    e[0, 1, 2] = e[1, 2, 0] = e[2, 0, 1] = 1.0
    e[0, 2, 1] = e[2, 1, 0] = e[1, 0, 2] = -1.0
    return e


def _assemble_kern(q_in, q_out, w_ss, w_vs, w_sv, w_vv0, w_vv1):
    offs = np.arange(K, dtype=np.float32) - (K - 1) / 2.0
    oz, oy, ox = np.meshgrid(offs, offs, offs, indexing='ij')
    p_off = np.stack([oz, oy, ox], -1).reshape(-1, 3)
    v = p_off[None, None] - (q_out[:, None, None] - q_in[None, :, None])
    r = np.linalg.norm(v, axis=-1)
    u = np.where(r[..., None] > EPS, v / np.maximum(r, EPS)[..., None], 0.0).astype(np.float32)
    centers = np.linspace(0.0, R_MAX, G).astype(np.float32)
    sigma = R_MAX / (G - 1)
    R = np.exp(-0.5 * ((r[..., None] - centers) / sigma) ** 2).astype(np.float32)
    RY = R[..., None] * u[..., None, :]
    eye3 = np.eye(3, dtype=np.float32)
    eps3 = _levi_civita()
    K_ss = np.einsum('acg,pqkg->apcqk', w_ss, R, optimize=True)
    K_vs = np.einsum('acg,pqkgm->ampcqk', w_vs, RY, optimize=True)
    K_sv = np.einsum('acg,pqkgm->apcmqk', w_sv, RY, optimize=True)
    K_vv = (np.einsum('acg,pqkg,mn->ampcnqk', w_vv0, R, eye3, optimize=True)
            + np.float32(0.7071067811865476) *
            np.einsum('acg,pqkgm,imj->aipcjqk', w_vv1, RY, eps3, optimize=True))
    Qo, Qi = q_out.shape[0], q_in.shape[0]
    top = np.concatenate([K_ss, K_sv.reshape(C0, Qo, 3 * C1, Qi, K3)], axis=2)
    bot = np.concatenate([K_vs.reshape(3 * C1, Qo, C0, Qi, K3),
                          K_vv.reshape(3 * C1, Qo, 3 * C1, Qi, K3)], axis=2)
    kern = np.concatenate([top, bot], axis=0)
    return np.ascontiguousarray(kern.reshape(DIM * Qo, DIM * Qi, K3).astype(np.float32))


# chunk schedule within one column: (x-tile index 0..4, dz window offset)
# tiles: 0=X1 ch0:128, 1=X2 ch128:256, 2=X3 mixed (ch256:320 | ch0:64 @z+1),
#        3=X4 ch64:192, 4=X5 ch192:320
_COL_SCHED = []
for _dz0 in (0, 2):
    _COL_SCHED += [(0, _dz0), (1, _dz0), (2, _dz0), (3, _dz0 + 1), (4, _dz0 + 1)]
_COL_SCHED += [(0, 4), (1, 4), (2, 4)]
assert len(_COL_SCHED) == NCPC


def _build_program():
    global _PROGRAM
    if _PROGRAM is not None:
        return _PROGRAM
    from contextlib import ExitStack
    from concourse import bass, mybir

    nc = bass.Bass("TRN2", target_bir_lowering=False, debug=False,
                   enable_asserts=False, num_devices=NCORES)
    ker_d = nc.dram_tensor("ker", [128, NCHUNK * NCH], mybir.dt.float16,
                           kind="ExternalInput").ap()
    xt_d = nc.dram_tensor("xt", [NSLOT, 5, 128, 768], mybir.dt.float16,
                          kind="ExternalInput").ap()
    xe_d = nc.dram_tensor("xe", [NEX, 128, NV], mybir.dt.float16,
                          kind="ExternalInput").ap()
    out_d = nc.dram_tensor("out_part", [128, NM * NCH], mybir.dt.float16,
                           kind="ExternalOutput").ap()

    # ker DMA groups: fine-grained so PE chunk waits are never far ahead
    KGRP = [(0, 3), (3, 5), (8, 5), (13, 5), (18, 5), (23, 5), (28, 5),
            (33, 5), (38, 3)]
    def kgrp_of(ci):
        for g, (c0, n) in enumerate(KGRP):
            if ci < c0 + n:
                return g
        raise AssertionError(ci)

    with ExitStack() as ctx:
        ktile = ctx.enter_context(nc.sbuf_tensor("kt", [128, NCHUNK * NCH],
                                                 mybir.dt.float16))
        xtiles = [[ctx.enter_context(nc.sbuf_tensor(f"x{s}_{i}", [128, 12, 8, 8],
                                                    mybir.dt.float16))
                   for i in range(5)] for s in range(NSLOT)]
        xex = [ctx.enter_context(nc.sbuf_tensor(f"xe{j}", [128, NV], mybir.dt.float16))
               for j in range(NEX)]
        otile = ctx.enter_context(nc.sbuf_tensor("otile", [128, NM * NCH],
                                                 mybir.dt.float16))
        wtile = ctx.enter_context(nc.sbuf_tensor("wtile", [128, 64], mybir.dt.float16))
        psum = [ctx.enter_context(nc.psum_tensor(f"psum{m}", [128, NCH], mybir.dt.float32))
                for m in range(NM)]
        ksem = ctx.enter_context(nc.semaphore("ksem"))
        xsems = [ctx.enter_context(nc.semaphore(f"xsem{s}")) for s in range(NSLOT + 1)]
        wsem = ctx.enter_context(nc.semaphore("wsem"))
        psem = ctx.enter_context(nc.semaphore("psem"))
        vsem = ctx.enter_context(nc.semaphore("vsem"))
        osem = ctx.enter_context(nc.semaphore("osem"))
        block = ctx.enter_context(nc.Block())

        def kgrp_dma(q, g):
            c0, nch_ = KGRP[g]
            q.dma_start(out=ktile[:, c0 * NCH:(c0 + nch_) * NCH],
                        in_=ker_d[:, c0 * NCH:(c0 + nch_) * NCH]
                        ).then_inc(ksem, 16)

        @block.sync
        def _(sync):
            sync.wait_ge(vsem, NM)
            sync.dma_start(out=out_d[:, :], in_=otile[:, :]).then_inc(osem, 16)

        @block.scalar
        def _(scalar):
            # single hand-ordered stream: ker chunk groups interleave x tiles
            # exactly in consumption order, so the PE never starves.
            g = 0
            kgrp_dma(scalar, g); g += 1
            for s in range(NSLOT):
                for i in range(5):
                    scalar.dma_start(out=xtiles[s][i][:, :, :, :],
                                     in_=xt_d[s, i, :, :]).then_inc(xsems[s], 16)
                    if i in (2, 4):
                        kgrp_dma(scalar, g); g += 1
            for j in range(NEX):
                scalar.dma_start(out=xex[j][:, :],
                                 in_=xe_d[j, :, :]).then_inc(xsems[NSLOT], 16)
            while g < len(KGRP):
                kgrp_dma(scalar, g); g += 1
            scalar.wait_ge(psem, 9)
            scalar.copy(otile[:, 2 * NCH:3 * NCH], psum[2][:, :]).then_inc(vsem, 1)
            scalar.wait_ge(psem, 10)
            scalar.copy(otile[:, 3 * NCH:], psum[3][:, :]).then_inc(vsem, 1)

        @block.tensor
        def _(tensor):
            tensor.wait_ge(wsem, 1)
            for w in range(WARMUP):
                tensor.matmul(psum[0][:8, :64], wtile[:, :8], wtile[:, :],
                              start=True, stop=True)
            ci = 0
            kwaited = 0
            for s in range(NSLOT):
                for j, (xi, dz) in enumerate(_COL_SCHED):
                    if j == 0:
                        tensor.wait_ge(xsems[s], 16 * 3)
                    elif j == 3:
                        tensor.wait_ge(xsems[s], 16 * 4)
                    elif j == 4:
                        tensor.wait_ge(xsems[s], 16 * 5)
                    need = kgrp_of(ci) + 1
                    if need > kwaited:
                        tensor.wait_ge(ksem, 16 * need)
                        kwaited = need
                    for m in range(NM):
                        tensor.matmul(
                            psum[m][:, :],
                            xtiles[s][xi][:, dz + 2 * m:dz + 2 * m + 2, :, :],
                            ktile[:, ci * NCH:(ci + 1) * NCH],
                            start=(ci == 0), stop=False)
                    ci += 1
            tensor.wait_ge(xsems[NSLOT], 16 * NEX)
            tensor.wait_ge(ksem, 16 * len(KGRP))
            for j in range(NEX):
                for m in range(NM):
                    mm = tensor.matmul(
                        psum[m][:, :],
                        xex[j][:, m * 128:(m + 1) * 128],
                        ktile[:, ci * NCH:(ci + 1) * NCH],
                        start=False, stop=(j == NEX - 1))
                    if j == NEX - 1:
                        mm.then_inc(psem, NM - m)
                ci += 1

        @block.vector
        def _(vector):
            vector.memset(wtile[:, :], 0.0).then_inc(wsem, 1)
            vector.wait_ge(psem, 4)
            vector.tensor_copy(otile[:, :NCH], psum[0][:, :]).then_inc(vsem, 1)
            vector.wait_ge(psem, 7)
            vector.tensor_copy(otile[:, NCH:2 * NCH], psum[1][:, :]).then_inc(vsem, 1)

        @block.gpsimd
        def _(gpsimd):
            gpsimd.wait_ge(osem, 16)

    _PROGRAM = nc
    return nc


def kernel(x, q_in, q_out, w_ss, w_vs, w_sv, w_vv0, w_vv1, bias):
    global LAST
    from concourse.bass_utils import run_bass_kernel_spmd

    kern = _assemble_kern(np.asarray(q_in, np.float32), np.asarray(q_out, np.float32),
                          np.asarray(w_ss, np.float32), np.asarray(w_vs, np.float32),
                          np.asarray(w_sv, np.float32), np.asarray(w_vv0, np.float32),
                          np.asarray(w_vv1, np.float32))
    xr = np.asarray(x, np.float32).reshape(NCH, P, P, P)
    x_pad = np.zeros((NCH, P + 4, P + 4, P + 4), np.float16)
    x_pad[:, 2:10, 2:10, 2:10] = xr.astype(np.float16)
    kerT = np.ascontiguousarray(kern.transpose(2, 1, 0)).astype(np.float16)  # (tap, C, A)

    in_maps = []
    for c in range(NCORES):
        cols = [c, 8 + c, 16 + c]
        xt = np.zeros((NSLOT, 5, 128, 768), np.float16)
        kchunks = np.zeros((NCHUNK, 128, NCH), np.float16)
        ci = 0
        for s, col in enumerate(cols):
            dy, dx = col // 5, col % 5
            w = x_pad[:, :, dy:dy + 8, dx:dx + 8]          # (320, 12, 8, 8)
            xt[s, 0] = w[0:128].reshape(128, 768)
            xt[s, 1] = w[128:256].reshape(128, 768)
            x3 = np.zeros((128, 12, 8, 8), np.float16)
            x3[:64] = w[256:320]
            x3[64:, :11] = w[0:64, 1:12]
            xt[s, 2] = x3.reshape(128, 768)
            xt[s, 3] = w[64:192].reshape(128, 768)
            xt[s, 4] = w[192:320].reshape(128, 768)
            for dz0 in (0, 2):
                tA, tB = 25 * dz0 + col, 25 * (dz0 + 1) + col
                kchunks[ci + 0] = kerT[tA, 0:128]
                kchunks[ci + 1] = kerT[tA, 128:256]
                kchunks[ci + 2, :64] = kerT[tA, 256:320]
                kchunks[ci + 2, 64:] = kerT[tB, 0:64]
                kchunks[ci + 3] = kerT[tB, 64:192]
                kchunks[ci + 4] = kerT[tB, 192:320]
                ci += 5
            t4 = 25 * 4 + col
            kchunks[ci + 0] = kerT[t4, 0:128]
            kchunks[ci + 1] = kerT[t4, 128:256]
            kchunks[ci + 2, :64] = kerT[t4, 256:320]
            ci += 3
        # column 24: rows stream (5 taps x 320 ch) -> 16 chunks, 2 per core
        col = 24
        dy, dx = col // 5, col % 5
        w24 = x_pad[:, :, dy:dy + 8, dx:dx + 8]
        xrows = np.zeros((5 * NCH, NV), np.float16)
        krows = np.zeros((5 * NCH, NCH), np.float16)
        for dz in range(5):
            t = 25 * dz + col
            xrows[dz * NCH:(dz + 1) * NCH] = \
                w24[:, dz:dz + 8].reshape(NCH, NV)
            krows[dz * NCH:(dz + 1) * NCH] = kerT[t]
        xe = np.zeros((NEX, 128, NV), np.float16)
        for j in range(NEX):
            r0 = (2 * c + j) * 128
            r1 = min(r0 + 128, 5 * NCH)
            if r0 < 5 * NCH:
                xe[j, :r1 - r0] = xrows[r0:r1]
                kchunks[ci, :r1 - r0] = krows[r0:r1]
            ci += 1
        assert ci == NCHUNK
        ker_c = np.ascontiguousarray(
            kchunks.transpose(1, 0, 2).reshape(128, NCHUNK * NCH))
        in_maps.append({"ker": ker_c, "xt": xt, "xe": xe})

    nc = _build_program()
    res = run_bass_kernel_spmd(nc, in_maps, list(range(NCORES)))
    LAST = res

    out = np.zeros((NV, NCH), np.float32)
    for c in range(NCORES):
        out += res.results[c]["out_part"].reshape(128, NM, NCH)\
            .transpose(1, 0, 2).reshape(NV, NCH).astype(np.float32)
    out = np.ascontiguousarray(out.T).reshape(1, DIM, Q, P, P, P)
    out[:, :C0] += np.asarray(bias, np.float32).reshape(1, C0, 1, 1, 1, 1)
    return out


# revision 3
# speedup vs baseline: 1.0048x; 1.0048x over previous
"""Equivariant PQ-layer conv kernel for 8x TRN2 NeuronCores — v3.

Sharding: 125 taps = 25 (dy,dx) columns x 5 dz. Cores own 3 columns each
(columns 0..23); column 24 is split across cores as 2 row-chunks each.
Within a column, the 5 taps differ only by a z-shift, so x ships as 5
per-core (dy,dx)-translated window tiles (128ch, 12z, 8y, 8x) that the PE
slices with program-constant 3D APs — no im2col duplication. Tap pairs
(dz,dz+1) chain the 320-row contraction into full 128-row K-chunks; the
chain's mixed chunk uses a tile whose upper 64 partitions hold ch 0:64
pre-shifted one z-slice. Transposed matmuls: psum[m] (128 vox, 320 A)
accumulates all 41 K-chunks; out partials are fp16, summed on host.
"""
import numpy as np

C0, C1 = 8, 4
K = 5
G = 8
EPS = 1e-6
R_MAX = 5.5
DIM = C0 + 3 * C1          # 20
Q = 16
P = 8
NCH = DIM * Q              # 320
NV = P * P * P             # 512
K3 = K ** 5 // K ** 2      # 125
NCORES = 8
NSLOT = 3                  # columns per core
NCPC = 13                  # K-chunks per column
NEX = 2                    # extra chunks (column 24 split)
NCHUNK = NSLOT * NCPC + NEX  # 41
NM = 4                     # vox M-chunks: 512/128
WARMUP = 72

LAST = None
_PROGRAM = None


def _levi_civita():
    e = np.zeros((3, 3, 3), np.float32)
    e[0, 1, 2] = e[1, 2, 0] = e[2, 0, 1] = 1.0
    e[0, 2, 1] = e[2, 1, 0] = e[1, 0, 2] = -1.0
    return e


def _assemble_kern(q_in, q_out, w_ss, w_vs, w_sv, w_vv0, w_vv1):
    offs = np.arange(K, dtype=np.float32) - (K - 1) / 2.0
    oz, oy, ox = np.meshgrid(offs, offs, offs, indexing='ij')
    p_off = np.stack([oz, oy, ox], -1).reshape(-1, 3)
    v = p_off[None, None] - (q_out[:, None, None] - q_in[None, :, None])
    r = np.linalg.norm(v, axis=-1)
    u = np.where(r[..., None] > EPS, v / np.maximum(r, EPS)[..., None], 0.0).astype(np.float32)
    centers = np.linspace(0.0, R_MAX, G).astype(np.float32)
    sigma = R_MAX / (G - 1)
    R = np.exp(-0.5 * ((r[..., None] - centers) / sigma) ** 2).astype(np.float32)
    RY = R[..., None] * u[..., None, :]
    eye3 = np.eye(3, dtype=np.float32)
    eps3 = _levi_civita()
    K_ss = np.einsum('acg,pqkg->apcqk', w_ss, R, optimize=True)
    K_vs = np.einsum('acg,pqkgm->ampcqk', w_vs, RY, optimize=True)
    K_sv = np.einsum('acg,pqkgm->apcmqk', w_sv, RY, optimize=True)
    K_vv = (np.einsum('acg,pqkg,mn->ampcnqk', w_vv0, R, eye3, optimize=True)
            + np.float32(0.7071067811865476) *
            np.einsum('acg,pqkgm,imj->aipcjqk', w_vv1, RY, eps3, optimize=True))
    Qo, Qi = q_out.shape[0], q_in.shape[0]
    top = np.concatenate([K_ss, K_sv.reshape(C0, Qo, 3 * C1, Qi, K3)], axis=2)
    bot = np.concatenate([K_vs.reshape(3 * C1, Qo, C0, Qi, K3),
                          K_vv.reshape(3 * C1, Qo, 3 * C1, Qi, K3)], axis=2)
    kern = np.concatenate([top, bot], axis=0)
    return np.ascontiguousarray(kern.reshape(DIM * Qo, DIM * Qi, K3).astype(np.float32))


# chunk schedule within one column: (x-tile index 0..4, dz window offset)
# tiles: 0=X1 ch0:128, 1=X2 ch128:256, 2=X3 mixed (ch256:320 | ch0:64 @z+1),
#        3=X4 ch64:192, 4=X5 ch192:320
_COL_SCHED = []
for _dz0 in (0, 2):
    _COL_SCHED += [(0, _dz0), (1, _dz0), (2, _dz0), (3, _dz0 + 1), (4, _dz0 + 1)]
_COL_SCHED += [(0, 4), (1, 4), (2, 4)]
assert len(_COL_SCHED) == NCPC


def _build_program():
    global _PROGRAM
    if _PROGRAM is not None:
        return _PROGRAM
    from contextlib import ExitStack
    from concourse import bass, mybir

    nc = bass.Bass("TRN2", target_bir_lowering=False, debug=False,
                   enable_asserts=False, num_devices=NCORES)
    ker_d = nc.dram_tensor("ker", [128, NCHUNK * NCH], mybir.dt.float16,
                           kind="ExternalInput").ap()
    xt_d = nc.dram_tensor("xt", [NSLOT, 5, 128, 768], mybir.dt.float16,
                          kind="ExternalInput").ap()
    xe_d = nc.dram_tensor("xe", [NEX, 128, NV], mybir.dt.float16,
                          kind="ExternalInput").ap()
    out_d = nc.dram_tensor("out_part", [128, NM * NCH], mybir.dt.float16,
                           kind="ExternalOutput").ap()

    # ker DMA groups: fine-grained so PE chunk waits are never far ahead
    KGRP = [(0, 3), (3, 5), (8, 5), (13, 5), (18, 5), (23, 5), (28, 5),
            (33, 5), (38, 3)]
    def kgrp_of(ci):
        for g, (c0, n) in enumerate(KGRP):
            if ci < c0 + n:
                return g
        raise AssertionError(ci)

    with ExitStack() as ctx:
        ktile = ctx.enter_context(nc.sbuf_tensor("kt", [128, NCHUNK * NCH],
                                                 mybir.dt.float16))
        xtiles = [[ctx.enter_context(nc.sbuf_tensor(f"x{s}_{i}", [128, 12, 8, 8],
                                                    mybir.dt.float16))
                   for i in range(5)] for s in range(NSLOT)]
        xex = [ctx.enter_context(nc.sbuf_tensor(f"xe{j}", [128, NV], mybir.dt.float16))
               for j in range(NEX)]
        otile = ctx.enter_context(nc.sbuf_tensor("otile", [128, NM * NCH],
                                                 mybir.dt.float16))
        wtile = ctx.enter_context(nc.sbuf_tensor("wtile", [128, 64], mybir.dt.float16))
        psum = [ctx.enter_context(nc.psum_tensor(f"psum{m}", [128, NCH], mybir.dt.float32))
                for m in range(NM)]
        kgsems = [ctx.enter_context(nc.semaphore(f"kg{g}")) for g in range(9)]
        xtsems = [[ctx.enter_context(nc.semaphore(f"xts{s}_{i}")) for i in range(5)]
                  for s in range(NSLOT)]
        xesems = [ctx.enter_context(nc.semaphore(f"xes{j}")) for j in range(NEX)]
        wsem = ctx.enter_context(nc.semaphore("wsem"))
        psem = ctx.enter_context(nc.semaphore("psem"))
        vsem = ctx.enter_context(nc.semaphore("vsem"))
        osem = ctx.enter_context(nc.semaphore("osem"))
        block = ctx.enter_context(nc.Block())

        def kgrp_dma(q, g):
            c0, nch_ = KGRP[g]
            q.dma_start(out=ktile[:, c0 * NCH:(c0 + nch_) * NCH],
                        in_=ker_d[:, c0 * NCH:(c0 + nch_) * NCH]
                        ).then_inc(kgsems[g], 16)

        @block.sync
        def _(sync):
            sync.wait_ge(vsem, NM)
            sync.dma_start(out=out_d[:, :], in_=otile[:, :]).then_inc(osem, 16)

        @block.scalar
        def _(scalar):
            # single hand-ordered stream: ker chunk groups interleave x tiles
            # exactly in consumption order, so the PE never starves.
            g = 0
            kgrp_dma(scalar, g); g += 1
            for s in range(NSLOT):
                for i in range(5):
                    scalar.dma_start(out=xtiles[s][i][:, :, :, :],
                                     in_=xt_d[s, i, :, :]).then_inc(xtsems[s][i], 16)
                    if i in (2, 4):
                        kgrp_dma(scalar, g); g += 1
            for j in range(NEX):
                scalar.dma_start(out=xex[j][:, :],
                                 in_=xe_d[j, :, :]).then_inc(xesems[j], 16)
            while g < len(KGRP):
                kgrp_dma(scalar, g); g += 1
            scalar.wait_ge(psem, 9)
            scalar.copy(otile[:, 2 * NCH:3 * NCH], psum[2][:, :]).then_inc(vsem, 1)
            scalar.wait_ge(psem, 10)
            scalar.copy(otile[:, 3 * NCH:], psum[3][:, :]).then_inc(vsem, 1)

        @block.tensor
        def _(tensor):
            tensor.wait_ge(wsem, 1)
            for w in range(WARMUP):
                tensor.matmul(psum[0][:8, :64], wtile[:, :8], wtile[:, :],
                              start=True, stop=True)
            ci = 0
            kwaited = 0
            xtwaited = set()
            for s in range(NSLOT):
                for j, (xi, dz) in enumerate(_COL_SCHED):
                    if (s, xi) not in xtwaited:
                        tensor.wait_ge(xtsems[s][xi], 16)
                        xtwaited.add((s, xi))
                    need = kgrp_of(ci) + 1
                    if need > kwaited:
                        for g in range(kwaited, need):
                            tensor.wait_ge(kgsems[g], 16)
                        kwaited = need
                    for m in range(NM):
                        tensor.matmul(
                            psum[m][:, :],
                            xtiles[s][xi][:, dz + 2 * m:dz + 2 * m + 2, :, :],
                            ktile[:, ci * NCH:(ci + 1) * NCH],
                            start=(ci == 0), stop=False)
                    ci += 1
            for j in range(NEX):
                tensor.wait_ge(xesems[j], 16)
            for g in range(kwaited, len(KGRP)):
                tensor.wait_ge(kgsems[g], 16)
            for j in range(NEX):
                for m in range(NM):
                    mm = tensor.matmul(
                        psum[m][:, :],
                        xex[j][:, m * 128:(m + 1) * 128],
                        ktile[:, ci * NCH:(ci + 1) * NCH],
                        start=False, stop=(j == NEX - 1))
                    if j == NEX - 1:
                        mm.then_inc(psem, NM - m)
                ci += 1

        @block.vector
        def _(vector):
            vector.memset(wtile[:, :], 0.0).then_inc(wsem, 1)
            vector.wait_ge(psem, 4)
            vector.tensor_copy(otile[:, :NCH], psum[0][:, :]).then_inc(vsem, 1)
            vector.wait_ge(psem, 7)
            vector.tensor_copy(otile[:, NCH:2 * NCH], psum[1][:, :]).then_inc(vsem, 1)

        @block.gpsimd
        def _(gpsimd):
            gpsimd.wait_ge(osem, 16)

    _PROGRAM = nc
    return nc


def kernel(x, q_in, q_out, w_ss, w_vs, w_sv, w_vv0, w_vv1, bias):
    global LAST
    from concourse.bass_utils import run_bass_kernel_spmd

    kern = _assemble_kern(np.asarray(q_in, np.float32), np.asarray(q_out, np.float32),
                          np.asarray(w_ss, np.float32), np.asarray(w_vs, np.float32),
                          np.asarray(w_sv, np.float32), np.asarray(w_vv0, np.float32),
                          np.asarray(w_vv1, np.float32))
    xr = np.asarray(x, np.float32).reshape(NCH, P, P, P)
    x_pad = np.zeros((NCH, P + 4, P + 4, P + 4), np.float16)
    x_pad[:, 2:10, 2:10, 2:10] = xr.astype(np.float16)
    kerT = np.ascontiguousarray(kern.transpose(2, 1, 0)).astype(np.float16)  # (tap, C, A)

    in_maps = []
    for c in range(NCORES):
        cols = [c, 8 + c, 16 + c]
        xt = np.zeros((NSLOT, 5, 128, 768), np.float16)
        kchunks = np.zeros((NCHUNK, 128, NCH), np.float16)
        ci = 0
        for s, col in enumerate(cols):
            dy, dx = col // 5, col % 5
            w = x_pad[:, :, dy:dy + 8, dx:dx + 8]          # (320, 12, 8, 8)
            xt[s, 0] = w[0:128].reshape(128, 768)
            xt[s, 1] = w[128:256].reshape(128, 768)
            x3 = np.zeros((128, 12, 8, 8), np.float16)
            x3[:64] = w[256:320]
            x3[64:, :11] = w[0:64, 1:12]
            xt[s, 2] = x3.reshape(128, 768)
            xt[s, 3] = w[64:192].reshape(128, 768)
            xt[s, 4] = w[192:320].reshape(128, 768)
            for dz0 in (0, 2):
                tA, tB = 25 * dz0 + col, 25 * (dz0 + 1) + col
                kchunks[ci + 0] = kerT[tA, 0:128]
                kchunks[ci + 1] = kerT[tA, 128:256]
                kchunks[ci + 2, :64] = kerT[tA, 256:320]
                kchunks[ci + 2, 64:] = kerT[tB, 0:64]
                kchunks[ci + 3] = kerT[tB, 64:192]
                kchunks[ci + 4] = kerT[tB, 192:320]
                ci += 5
            t4 = 25 * 4 + col
            kchunks[ci + 0] = kerT[t4, 0:128]
            kchunks[ci + 1] = kerT[t4, 128:256]
            kchunks[ci + 2, :64] = kerT[t4, 256:320]
            ci += 3
        # column 24: rows stream (5 taps x 320 ch) -> 16 chunks, 2 per core
        col = 24
        dy, dx = col // 5, col % 5
        w24 = x_pad[:, :, dy:dy + 8, dx:dx + 8]
        xrows = np.zeros((5 * NCH, NV), np.float16)
        krows = np.zeros((5 * NCH, NCH), np.float16)
        for dz in range(5):
            t = 25 * dz + col
            xrows[dz * NCH:(dz + 1) * NCH] = \
                w24[:, dz:dz + 8].reshape(NCH, NV)
            krows[dz * NCH:(dz + 1) * NCH] = kerT[t]
        xe = np.zeros((NEX, 128, NV), np.float16)
        for j in range(NEX):
            r0 = (2 * c + j) * 128
            r1 = min(r0 + 128, 5 * NCH)
            if r0 < 5 * NCH:
                xe[j, :r1 - r0] = xrows[r0:r1]
                kchunks[ci, :r1 - r0] = krows[r0:r1]
            ci += 1
        assert ci == NCHUNK
        ker_c = np.ascontiguousarray(
            kchunks.transpose(1, 0, 2).reshape(128, NCHUNK * NCH))
        in_maps.append({"ker": ker_c, "xt": xt, "xe": xe})

    nc = _build_program()
    res = run_bass_kernel_spmd(nc, in_maps, list(range(NCORES)))
    LAST = res

    out = np.zeros((NV, NCH), np.float32)
    for c in range(NCORES):
        out += res.results[c]["out_part"].reshape(128, NM, NCH)\
            .transpose(1, 0, 2).reshape(NV, NCH).astype(np.float32)
    out = np.ascontiguousarray(out.T).reshape(1, DIM, Q, P, P, P)
    out[:, :C0] += np.asarray(bias, np.float32).reshape(1, C0, 1, 1, 1, 1)
    return out


# revision 4
# speedup vs baseline: 1.0129x; 1.0080x over previous
"""Equivariant PQ-layer conv kernel for 8x TRN2 NeuronCores — v4.

Sharding: 125 taps = 25 (dy,dx) columns x 5 dz. Cores own 3 columns each
(columns 0..23); column 24 is split across cores as 2 row-chunks each.
Within a column, the 5 taps differ only by a z-shift, so x ships as 5
per-core (dy,dx)-translated window tiles (128ch, 12z, 8y, 8x) that the PE
slices with program-constant 3D APs — no im2col duplication. Tap pairs
(dz,dz+1) chain the 320-row contraction into full 128-row K-chunks; the
dz=4 singles chain across the core's 3 columns via a cross-column mixed
tile, giving 40 K-chunks total. Transposed matmuls: psum[m] (128 vox,
320 A) accumulates all chunks at full PE clock (warmup matmuls ride out
the p-state ramp during the initial DMA); out partials are fp16, summed
on host. All semaphore waits are exact full counts on dedicated sems —
DMA completions can reorder across the 16 engines, so k-of-n threshold
waits on a shared sem are racy.
"""
import numpy as np

C0, C1 = 8, 4
K = 5
G = 8
EPS = 1e-6
R_MAX = 5.5
DIM = C0 + 3 * C1          # 20
Q = 16
P = 8
NCH = DIM * Q              # 320
NV = P * P * P             # 512
K3 = K ** 5 // K ** 2      # 125
NCORES = 8
NSLOT = 3                  # columns per core
NEX = 2                    # extra chunks (column 24 split)
NCHUNK = 40                # 12 + 13 + 13 + 2
NM = 4
WARMUP = 56

LAST = None
_PROGRAM = None


def _levi_civita():
    e = np.zeros((3, 3, 3), np.float32)
    e[0, 1, 2] = e[1, 2, 0] = e[2, 0, 1] = 1.0
    e[0, 2, 1] = e[2, 1, 0] = e[1, 0, 2] = -1.0
    return e


def _assemble_kern(q_in, q_out, w_ss, w_vs, w_sv, w_vv0, w_vv1):
    offs = np.arange(K, dtype=np.float32) - (K - 1) / 2.0
    oz, oy, ox = np.meshgrid(offs, offs, offs, indexing='ij')
    p_off = np.stack([oz, oy, ox], -1).reshape(-1, 3)
    v = p_off[None, None] - (q_out[:, None, None] - q_in[None, :, None])
    r = np.linalg.norm(v, axis=-1)
    u = np.where(r[..., None] > EPS, v / np.maximum(r, EPS)[..., None], 0.0).astype(np.float32)
    centers = np.linspace(0.0, R_MAX, G).astype(np.float32)
    sigma = R_MAX / (G - 1)
    R = np.exp(-0.5 * ((r[..., None] - centers) / sigma) ** 2).astype(np.float32)
    RY = R[..., None] * u[..., None, :]
    eye3 = np.eye(3, dtype=np.float32)
    eps3 = _levi_civita()
    K_ss = np.einsum('acg,pqkg->apcqk', w_ss, R, optimize=True)
    K_vs = np.einsum('acg,pqkgm->ampcqk', w_vs, RY, optimize=True)
    K_sv = np.einsum('acg,pqkgm->apcmqk', w_sv, RY, optimize=True)
    K_vv = (np.einsum('acg,pqkg,mn->ampcnqk', w_vv0, R, eye3, optimize=True)
            + np.float32(0.7071067811865476) *
            np.einsum('acg,pqkgm,imj->aipcjqk', w_vv1, RY, eps3, optimize=True))
    Qo, Qi = q_out.shape[0], q_in.shape[0]
    top = np.concatenate([K_ss, K_sv.reshape(C0, Qo, 3 * C1, Qi, K3)], axis=2)
    bot = np.concatenate([K_vs.reshape(3 * C1, Qo, C0, Qi, K3),
                          K_vv.reshape(3 * C1, Qo, 3 * C1, Qi, K3)], axis=2)
    kern = np.concatenate([top, bot], axis=0)
    return np.ascontiguousarray(kern.reshape(DIM * Qo, DIM * Qi, K3).astype(np.float32))


# chunk schedule within one column: (x-tile index 0..4, dz window offset)
# tiles: 0=X1 ch0:128, 1=X2 ch128:256, 2=X3 mixed (ch256:320 | ch0:64 @z+1),
#        3=X4 ch64:192, 4=X5 ch192:320
def _mk_slot_scheds():
    pairs = []
    for dz0 in (0, 2):
        pairs += [(0, dz0, False), (1, dz0, False), (2, dz0, False),
                  (3, dz0 + 1, False), (4, dz0 + 1, False)]
    s0 = [pairs[0], (0, 4, True), pairs[1], (1, 4, True), pairs[2],
          pairs[5], pairs[3], pairs[4]] + pairs[6:]
    s1 = pairs + [(3, 4, True), (4, 4, True), (5, 4, True)]
    s2 = pairs + [(0, 4, True), (1, 4, True), (2, 4, True)]
    return [s0, s1, s2]

_SLOT_SCHEDS = _mk_slot_scheds()
assert sum(len(s) for s in _SLOT_SCHEDS) + NEX == NCHUNK


def _build_program():
    global _PROGRAM
    if _PROGRAM is not None:
        return _PROGRAM
    from contextlib import ExitStack
    from concourse import bass, mybir

    nc = bass.Bass("TRN2", target_bir_lowering=False, debug=False,
                   enable_asserts=False, num_devices=NCORES)
    ker_d = nc.dram_tensor("ker", [128, NCHUNK * NCH], mybir.dt.float16,
                           kind="ExternalInput").ap()
    xt_d = nc.dram_tensor("xt", [NSLOT * 5 + 1, 128, 768], mybir.dt.float16,
                          kind="ExternalInput").ap()
    xe_d = nc.dram_tensor("xe", [NEX, 128, NV], mybir.dt.float16,
                          kind="ExternalInput").ap()
    out_d = nc.dram_tensor("out_part", [128, NM * NCH], mybir.dt.float16,
                           kind="ExternalOutput").ap()

    # ker DMA groups: fine-grained so PE chunk waits are never far ahead
    KGRP = [(0, 1), (1, 2), (3, 4), (7, 5), (12, 5), (17, 5), (22, 3),
            (25, 5), (30, 5), (35, 3), (38, 2)]
    def kgrp_of(ci):
        for g, (c0, n) in enumerate(KGRP):
            if ci < c0 + n:
                return g
        raise AssertionError(ci)

    # 1-based positions of each resource in the scalar DMA stream;
    # chunk ci is runnable once dsem >= 16 * NEED[ci].
    kpos = {}
    xpos = {}
    _order = [('kg', 0), ('x', 0, 0), ('kg', 1), ('x', 0, 1), ('x', 0, 2),
          ('kg', 2), ('x', 0, 3), ('x', 0, 4), ('kg', 3), ('x', 1, 0),
          ('x', 1, 1), ('kg', 4), ('x', 1, 2), ('x', 1, 3), ('kg', 5),
          ('x', 1, 4), ('xmab',), ('kg', 6), ('x', 2, 0), ('x', 2, 1),
          ('kg', 7), ('x', 2, 2), ('x', 2, 3), ('kg', 8), ('x', 2, 4),
          ('kg', 9), ('xe', 0), ('xe', 1), ('kg', 10)]
    for pos, ent in enumerate(_order, 1):
        if ent[0] == 'kg':
            c0, n = KGRP[ent[1]]
            for ch in range(c0, c0 + n):
                kpos[ch] = pos
        elif ent[0] == 'x':
            xpos[(ent[1], ent[2])] = pos
        elif ent[0] == 'xmab':
            xpos['mab'] = pos
        else:
            xpos[('e', ent[1])] = pos


    with ExitStack() as ctx:
        ktile = ctx.enter_context(nc.sbuf_tensor("kt", [128, NCHUNK * NCH],
                                                 mybir.dt.float16))
        xtiles = [[ctx.enter_context(nc.sbuf_tensor(f"x{s}_{i}", [128, 12, 8, 8],
                                                    mybir.dt.float16))
                   for i in range(5)] for s in range(NSLOT)]
        xmab = ctx.enter_context(nc.sbuf_tensor("xmab", [128, 12, 8, 8],
                                                mybir.dt.float16))
        xex = [ctx.enter_context(nc.sbuf_tensor(f"xe{j}", [128, NV], mybir.dt.float16))
               for j in range(NEX)]
        otile = ctx.enter_context(nc.sbuf_tensor("otile", [128, NM * NCH],
                                                 mybir.dt.float16))
        wtile = ctx.enter_context(nc.sbuf_tensor("wtile", [128, 64], mybir.dt.float16))
        psum = [ctx.enter_context(nc.psum_tensor(f"psum{m}", [128, NCH], mybir.dt.float32))
                for m in range(NM)]
        kgsems = [ctx.enter_context(nc.semaphore(f"kg{g}")) for g in range(len(KGRP))]
        xtsems = [[ctx.enter_context(nc.semaphore(f"xts{s}_{i}")) for i in range(5)]
                  for s in range(NSLOT)]
        xmsem = ctx.enter_context(nc.semaphore("xmsem"))
        xesems = [ctx.enter_context(nc.semaphore(f"xes{j}")) for j in range(NEX)]
        wsem = ctx.enter_context(nc.semaphore("wsem"))
        psem = ctx.enter_context(nc.semaphore("psem"))
        vsemA = ctx.enter_context(nc.semaphore("vsemA"))
        vsemB = ctx.enter_context(nc.semaphore("vsemB"))
        osem = ctx.enter_context(nc.semaphore("osem"))
        block = ctx.enter_context(nc.Block())

        def kgrp_dma(q, g):
            c0, nch_ = KGRP[g]
            q.dma_start(out=ktile[:, c0 * NCH:(c0 + nch_) * NCH],
                        in_=ker_d[:, c0 * NCH:(c0 + nch_) * NCH]
                        ).then_inc(kgsems[g], 16)

        @block.sync
        def _(sync):
            sync.wait_ge(vsemA, 1)
            sync.dma_start(out=out_d[:, :NCH],
                           in_=otile[:, :NCH]).then_inc(osem, 16)
            sync.wait_ge(vsemA, 2)
            sync.dma_start(out=out_d[:, NCH:2 * NCH],
                           in_=otile[:, NCH:2 * NCH]).then_inc(osem, 16)

        @block.scalar
        def _(scalar):
            # single hand-ordered stream: ker chunk groups interleave x tiles
            # exactly in consumption order, so the PE never starves.
            def xdma(scalar, s, i):
                scalar.dma_start(out=xtiles[s][i][:, :, :, :],
                                 in_=xt_d[5 * s + i, :, :]).then_inc(xtsems[s][i], 16)
            kgrp_dma(scalar, 0)
            xdma(scalar, 0, 0)
            kgrp_dma(scalar, 1)
            xdma(scalar, 0, 1)
            xdma(scalar, 0, 2)
            kgrp_dma(scalar, 2)
            xdma(scalar, 0, 3)
            xdma(scalar, 0, 4)
            kgrp_dma(scalar, 3)
            xdma(scalar, 1, 0)
            xdma(scalar, 1, 1)
            kgrp_dma(scalar, 4)
            xdma(scalar, 1, 2)
            xdma(scalar, 1, 3)
            kgrp_dma(scalar, 5)
            xdma(scalar, 1, 4)
            scalar.dma_start(out=xmab[:, :, :, :],
                             in_=xt_d[15, :, :]).then_inc(xmsem, 16)
            kgrp_dma(scalar, 6)
            xdma(scalar, 2, 0)
            xdma(scalar, 2, 1)
            kgrp_dma(scalar, 7)
            xdma(scalar, 2, 2)
            xdma(scalar, 2, 3)
            kgrp_dma(scalar, 8)
            xdma(scalar, 2, 4)
            kgrp_dma(scalar, 9)
            for j in range(NEX):
                scalar.dma_start(out=xex[j][:, :],
                                 in_=xe_d[j, :, :]).then_inc(xesems[j], 16)
            kgrp_dma(scalar, 10)
            scalar.wait_ge(psem, 9)
            scalar.copy(otile[:, 2 * NCH:3 * NCH], psum[2][:, :]).then_inc(vsemB, 1)
            scalar.wait_ge(psem, 10)
            scalar.copy(otile[:, 3 * NCH:], psum[3][:, :]).then_inc(vsemB, 1)
            scalar.wait_ge(vsemB, 2)
            scalar.dma_start(out=out_d[:, 2 * NCH:],
                             in_=otile[:, 2 * NCH:]).then_inc(osem, 16)

        @block.tensor
        def _(tensor):
            tensor.wait_ge(wsem, 1)
            for w in range(WARMUP):
                tensor.matmul(psum[0][:8, :64], wtile[:, :8], wtile[:, :],
                              start=True, stop=True)
            ci = 0
            kwaited = 0
            xtwaited = set()

            def kwait(ci):
                nonlocal kwaited
                need = kgrp_of(ci) + 1
                while kwaited < need:
                    tensor.wait_ge(kgsems[kwaited], 16)
                    kwaited += 1

            for s in range(NSLOT):
                for (xi, dz, sing) in _SLOT_SCHEDS[s]:
                    if xi == 5:
                        if 'mab' not in xtwaited:
                            tensor.wait_ge(xmsem, 16)
                            xtwaited.add('mab')
                    elif (s, xi) not in xtwaited:
                        tensor.wait_ge(xtsems[s][xi], 16)
                        xtwaited.add((s, xi))
                    kwait(ci)
                    src_tile = xmab if xi == 5 else xtiles[s][xi]
                    for m in range(NM):
                        tensor.matmul(
                            psum[m][:, :],
                            src_tile[:, dz + 2 * m:dz + 2 * m + 2, :, :],
                            ktile[:, ci * NCH:(ci + 1) * NCH],
                            start=(ci == 0), stop=False)
                    ci += 1
            for j in range(NEX):
                tensor.wait_ge(xesems[j], 16)
                kwait(ci)
                for m in range(NM):
                    mm = tensor.matmul(
                        psum[m][:, :],
                        xex[j][:, m * 128:(m + 1) * 128],
                        ktile[:, ci * NCH:(ci + 1) * NCH],
                        start=False, stop=(j == NEX - 1))
                    if j == NEX - 1:
                        mm.then_inc(psem, NM - m)
                ci += 1

        @block.vector
        def _(vector):
            vector.memset(wtile[:, :], 0.0).then_inc(wsem, 1)
            vector.wait_ge(psem, 4)
            vector.tensor_copy(otile[:, :NCH], psum[0][:, :]).then_inc(vsemA, 1)
            vector.wait_ge(psem, 7)
            vector.tensor_copy(otile[:, NCH:2 * NCH], psum[1][:, :]).then_inc(vsemA, 1)

        @block.gpsimd
        def _(gpsimd):
            gpsimd.wait_ge(osem, 48)

    _PROGRAM = nc
    return nc


def kernel(x, q_in, q_out, w_ss, w_vs, w_sv, w_vv0, w_vv1, bias):
    global LAST
    from concourse.bass_utils import run_bass_kernel_spmd

    kern = _assemble_kern(np.asarray(q_in, np.float32), np.asarray(q_out, np.float32),
                          np.asarray(w_ss, np.float32), np.asarray(w_vs, np.float32),
                          np.asarray(w_sv, np.float32), np.asarray(w_vv0, np.float32),
                          np.asarray(w_vv1, np.float32))
    xr = np.asarray(x, np.float32).reshape(NCH, P, P, P)
    x_pad = np.zeros((NCH, P + 4, P + 4, P + 4), np.float16)
    x_pad[:, 2:10, 2:10, 2:10] = xr.astype(np.float16)
    kerT = np.ascontiguousarray(kern.transpose(2, 1, 0)).astype(np.float16)  # (tap, C, A)

    in_maps = []
    for c in range(NCORES):
        cols = [c, 8 + c, 16 + c]
        xt = np.zeros((NSLOT * 5 + 1, 128, 768), np.float16)
        kchunks = np.zeros((NCHUNK, 128, NCH), np.float16)
        wins = []
        for s, col in enumerate(cols):
            dy, dx = col // 5, col % 5
            w = x_pad[:, :, dy:dy + 8, dx:dx + 8]          # (320, 12, 8, 8)
            wins.append(w)
            xt[5 * s + 0] = w[0:128].reshape(128, 768)
            xt[5 * s + 1] = w[128:256].reshape(128, 768)
            x3 = np.zeros((128, 12, 8, 8), np.float16)
            x3[:64] = w[256:320]
            x3[64:, :11] = w[0:64, 1:12]
            xt[5 * s + 2] = x3.reshape(128, 768)
            xt[5 * s + 3] = w[64:192].reshape(128, 768)
            xt[5 * s + 4] = w[192:320].reshape(128, 768)
        xm = np.zeros((128, 12, 8, 8), np.float16)
        xm[:64] = wins[0][256:320]
        xm[64:] = wins[1][0:64]
        xt[15] = xm.reshape(128, 768)

        ci = 0
        for s, col in enumerate(cols):
            def tap(d, col=col):
                return 25 * d + col
            for (xi, dz, sing) in _SLOT_SCHEDS[s]:
                if xi == 0:
                    kchunks[ci] = kerT[tap(dz), 0:128]
                elif xi == 1:
                    kchunks[ci] = kerT[tap(dz), 128:256]
                elif xi == 2 and sing:
                    kchunks[ci, :64] = kerT[tap(4), 256:320]
                elif xi == 2:
                    kchunks[ci, :64] = kerT[tap(dz), 256:320]
                    kchunks[ci, 64:] = kerT[tap(dz + 1), 0:64]
                elif xi == 3:
                    kchunks[ci] = kerT[tap(dz), 64:192]
                elif xi == 4:
                    kchunks[ci] = kerT[tap(dz), 192:320]
                elif xi == 5:
                    kchunks[ci, :64] = kerT[25 * 4 + cols[0], 256:320]
                    kchunks[ci, 64:] = kerT[25 * 4 + cols[1], 0:64]
                ci += 1
        # column 24: rows stream (5 taps x 320 ch) -> 16 chunks, 2 per core
        col = 24
        dy, dx = col // 5, col % 5
        w24 = x_pad[:, :, dy:dy + 8, dx:dx + 8]
        xrows = np.zeros((5 * NCH, NV), np.float16)
        krows = np.zeros((5 * NCH, NCH), np.float16)
        for dz in range(5):
            t = 25 * dz + col
            xrows[dz * NCH:(dz + 1) * NCH] = \
                w24[:, dz:dz + 8].reshape(NCH, NV)
            krows[dz * NCH:(dz + 1) * NCH] = kerT[t]
        xe = np.zeros((NEX, 128, NV), np.float16)
        for j in range(NEX):
            r0 = (2 * c + j) * 128
            r1 = min(r0 + 128, 5 * NCH)
            if r0 < 5 * NCH:
                xe[j, :r1 - r0] = xrows[r0:r1]
                kchunks[ci, :r1 - r0] = krows[r0:r1]
            ci += 1
        assert ci == NCHUNK, ci
        ker_c = np.ascontiguousarray(
            kchunks.transpose(1, 0, 2).reshape(128, NCHUNK * NCH))
        in_maps.append({"ker": ker_c, "xt": xt, "xe": xe})

    nc = _build_program()
    res = run_bass_kernel_spmd(nc, in_maps, list(range(NCORES)))
    LAST = res

    out = np.zeros((NV, NCH), np.float32)
    for c in range(NCORES):
        out += res.results[c]["out_part"].reshape(128, NM, NCH)\
            .transpose(1, 0, 2).reshape(NV, NCH).astype(np.float32)
    out = np.ascontiguousarray(out.T).reshape(1, DIM, Q, P, P, P)
    out[:, :C0] += np.asarray(bias, np.float32).reshape(1, C0, 1, 1, 1, 1)
    return out


# revision 5
# speedup vs baseline: 1.0161x; 1.0031x over previous
"""Equivariant PQ-layer conv kernel for 8x TRN2 NeuronCores — v4.

Sharding: 125 taps = 25 (dy,dx) columns x 5 dz. Cores own 3 columns each
(columns 0..23); column 24 is split across cores as 2 row-chunks each.
Within a column, the 5 taps differ only by a z-shift, so x ships as 5
per-core (dy,dx)-translated window tiles (128ch, 12z, 8y, 8x) that the PE
slices with program-constant 3D APs — no im2col duplication. Tap pairs
(dz,dz+1) chain the 320-row contraction into full 128-row K-chunks; the
dz=4 singles chain across the core's 3 columns via a cross-column mixed
tile, giving 40 K-chunks total. Transposed matmuls: psum[m] (128 vox,
320 A) accumulates all chunks at full PE clock (warmup matmuls ride out
the p-state ramp during the initial DMA); out partials are fp16, summed
on host. All semaphore waits are exact full counts on dedicated sems —
DMA completions can reorder across the 16 engines, so k-of-n threshold
waits on a shared sem are racy.
"""
import numpy as np

C0, C1 = 8, 4
K = 5
G = 8
EPS = 1e-6
R_MAX = 5.5
DIM = C0 + 3 * C1          # 20
Q = 16
P = 8
NCH = DIM * Q              # 320
NV = P * P * P             # 512
K3 = K ** 5 // K ** 2      # 125
NCORES = 8
NSLOT = 3                  # columns per core
NEX = 2                    # extra chunks (column 24 split)
NCHUNK = 40                # 12 + 13 + 13 + 2
NM = 4
WARMUP = 56

LAST = None
_PROGRAM = None


def _levi_civita():
    e = np.zeros((3, 3, 3), np.float32)
    e[0, 1, 2] = e[1, 2, 0] = e[2, 0, 1] = 1.0
    e[0, 2, 1] = e[2, 1, 0] = e[1, 0, 2] = -1.0
    return e


def _assemble_kern(q_in, q_out, w_ss, w_vs, w_sv, w_vv0, w_vv1):
    offs = np.arange(K, dtype=np.float32) - (K - 1) / 2.0
    oz, oy, ox = np.meshgrid(offs, offs, offs, indexing='ij')
    p_off = np.stack([oz, oy, ox], -1).reshape(-1, 3)
    v = p_off[None, None] - (q_out[:, None, None] - q_in[None, :, None])
    r = np.linalg.norm(v, axis=-1)
    u = np.where(r[..., None] > EPS, v / np.maximum(r, EPS)[..., None], 0.0).astype(np.float32)
    centers = np.linspace(0.0, R_MAX, G).astype(np.float32)
    sigma = R_MAX / (G - 1)
    R = np.exp(-0.5 * ((r[..., None] - centers) / sigma) ** 2).astype(np.float32)
    RY = R[..., None] * u[..., None, :]
    eye3 = np.eye(3, dtype=np.float32)
    eps3 = _levi_civita()
    K_ss = np.einsum('acg,pqkg->apcqk', w_ss, R, optimize=True)
    K_vs = np.einsum('acg,pqkgm->ampcqk', w_vs, RY, optimize=True)
    K_sv = np.einsum('acg,pqkgm->apcmqk', w_sv, RY, optimize=True)
    K_vv = (np.einsum('acg,pqkg,mn->ampcnqk', w_vv0, R, eye3, optimize=True)
            + np.float32(0.7071067811865476) *
            np.einsum('acg,pqkgm,imj->aipcjqk', w_vv1, RY, eps3, optimize=True))
    Qo, Qi = q_out.shape[0], q_in.shape[0]
    top = np.concatenate([K_ss, K_sv.reshape(C0, Qo, 3 * C1, Qi, K3)], axis=2)
    bot = np.concatenate([K_vs.reshape(3 * C1, Qo, C0, Qi, K3),
                          K_vv.reshape(3 * C1, Qo, 3 * C1, Qi, K3)], axis=2)
    kern = np.concatenate([top, bot], axis=0)
    return np.ascontiguousarray(kern.reshape(DIM * Qo, DIM * Qi, K3).astype(np.float32))


# chunk schedule within one column: (x-tile index 0..4, dz window offset)
# tiles: 0=X1 ch0:128, 1=X2 ch128:256, 2=X3 mixed (ch256:320 | ch0:64 @z+1),
#        3=X4 ch64:192, 4=X5 ch192:320
def _mk_slot_scheds():
    pairs = []
    for dz0 in (0, 2):
        pairs += [(0, dz0, False), (1, dz0, False), (2, dz0, False),
                  (3, dz0 + 1, False), (4, dz0 + 1, False)]
    s0 = [pairs[0], (0, 4, True), pairs[1], (1, 4, True), pairs[2],
          pairs[5], pairs[3], pairs[4]] + pairs[6:]
    s1 = pairs + [(3, 4, True), (4, 4, True), (5, 4, True)]
    s2 = pairs + [(0, 4, True), (1, 4, True), (2, 4, True)]
    return [s0, s1, s2]

_SLOT_SCHEDS = _mk_slot_scheds()
assert sum(len(s) for s in _SLOT_SCHEDS) + NEX == NCHUNK


def _build_program():
    global _PROGRAM
    if _PROGRAM is not None:
        return _PROGRAM
    from contextlib import ExitStack
    from concourse import bass, mybir

    nc = bass.Bass("TRN2", target_bir_lowering=False, debug=False,
                   enable_asserts=False, num_devices=NCORES)
    ker_d = nc.dram_tensor("ker", [128, NCHUNK * NCH], mybir.dt.float16,
                           kind="ExternalInput").ap()
    xt_d = nc.dram_tensor("xt", [NSLOT * 5 + 1, 128, 768], mybir.dt.float16,
                          kind="ExternalInput").ap()
    xe_d = nc.dram_tensor("xe", [NEX, 128, NV], mybir.dt.float16,
                          kind="ExternalInput").ap()
    out_d = nc.dram_tensor("out_part", [128, NM * NCH], mybir.dt.float16,
                           kind="ExternalOutput").ap()

    # ker DMA groups: fine-grained so PE chunk waits are never far ahead
    KGRP = [(0, 1), (1, 2), (3, 4), (7, 5), (12, 5), (17, 5), (22, 3),
            (25, 5), (30, 5), (35, 3), (38, 2)]
    def kgrp_of(ci):
        for g, (c0, n) in enumerate(KGRP):
            if ci < c0 + n:
                return g
        raise AssertionError(ci)

    # 1-based positions of each resource in the scalar DMA stream;
    # chunk ci is runnable once dsem >= 16 * NEED[ci].
    kpos = {}
    xpos = {}
    _order = [('kg', 0), ('x', 0, 0), ('kg', 1), ('x', 0, 1), ('x', 0, 2),
          ('kg', 2), ('x', 0, 3), ('x', 0, 4), ('kg', 3), ('x', 1, 0),
          ('x', 1, 1), ('kg', 4), ('x', 1, 2), ('x', 1, 3), ('kg', 5),
          ('x', 1, 4), ('xmab',), ('kg', 6), ('x', 2, 0), ('x', 2, 1),
          ('kg', 7), ('x', 2, 2), ('x', 2, 3), ('kg', 8), ('x', 2, 4),
          ('kg', 9), ('xe', 0), ('xe', 1), ('kg', 10)]
    for pos, ent in enumerate(_order, 1):
        if ent[0] == 'kg':
            c0, n = KGRP[ent[1]]
            for ch in range(c0, c0 + n):
                kpos[ch] = pos
        elif ent[0] == 'x':
            xpos[(ent[1], ent[2])] = pos
        elif ent[0] == 'xmab':
            xpos['mab'] = pos
        else:
            xpos[('e', ent[1])] = pos


    with ExitStack() as ctx:
        ktile = ctx.enter_context(nc.sbuf_tensor("kt", [128, NCHUNK * NCH],
                                                 mybir.dt.float16))
        xtiles = [[ctx.enter_context(nc.sbuf_tensor(f"x{s}_{i}", [128, 12, 8, 8],
                                                    mybir.dt.float16))
                   for i in range(5)] for s in range(NSLOT)]
        xmab = ctx.enter_context(nc.sbuf_tensor("xmab", [128, 12, 8, 8],
                                                mybir.dt.float16))
        xex = [ctx.enter_context(nc.sbuf_tensor(f"xe{j}", [128, NV], mybir.dt.float16))
               for j in range(NEX)]
        otile = ctx.enter_context(nc.sbuf_tensor("otile", [128, NM * NCH],
                                                 mybir.dt.float16))
        wtile = ctx.enter_context(nc.sbuf_tensor("wtile", [128, 64], mybir.dt.float16))
        psum = [ctx.enter_context(nc.psum_tensor(f"psum{m}", [128, NCH], mybir.dt.float32))
                for m in range(NM)]
        kgsems = [ctx.enter_context(nc.semaphore(f"kg{g}")) for g in range(len(KGRP))]
        xtsems = [[ctx.enter_context(nc.semaphore(f"xts{s}_{i}")) for i in range(5)]
                  for s in range(NSLOT)]
        xmsem = ctx.enter_context(nc.semaphore("xmsem"))
        xesems = [ctx.enter_context(nc.semaphore(f"xes{j}")) for j in range(NEX)]
        wsem = ctx.enter_context(nc.semaphore("wsem"))
        psem = ctx.enter_context(nc.semaphore("psem"))
        vsemA = ctx.enter_context(nc.semaphore("vsemA"))
        vsemB = ctx.enter_context(nc.semaphore("vsemB"))
        osem = ctx.enter_context(nc.semaphore("osem"))
        block = ctx.enter_context(nc.Block())

        def kgrp_dma(q, g):
            c0, nch_ = KGRP[g]
            q.dma_start(out=ktile[:, c0 * NCH:(c0 + nch_) * NCH],
                        in_=ker_d[:, c0 * NCH:(c0 + nch_) * NCH]
                        ).then_inc(kgsems[g], 16)

        @block.sync
        def _(sync):
            sync.wait_ge(vsemA, 2)
            sync.dma_start(out=out_d[:, :2 * NCH],
                           in_=otile[:, :2 * NCH]).then_inc(osem, 16)

        @block.scalar
        def _(scalar):
            # single hand-ordered stream: ker chunk groups interleave x tiles
            # exactly in consumption order, so the PE never starves.
            def xdma(scalar, s, i):
                scalar.dma_start(out=xtiles[s][i][:, :, :, :],
                                 in_=xt_d[5 * s + i, :, :]).then_inc(xtsems[s][i], 16)
            kgrp_dma(scalar, 0)
            xdma(scalar, 0, 0)
            kgrp_dma(scalar, 1)
            xdma(scalar, 0, 1)
            xdma(scalar, 0, 2)
            kgrp_dma(scalar, 2)
            xdma(scalar, 0, 3)
            xdma(scalar, 0, 4)
            kgrp_dma(scalar, 3)
            xdma(scalar, 1, 0)
            xdma(scalar, 1, 1)
            kgrp_dma(scalar, 4)
            xdma(scalar, 1, 2)
            xdma(scalar, 1, 3)
            kgrp_dma(scalar, 5)
            xdma(scalar, 1, 4)
            scalar.dma_start(out=xmab[:, :, :, :],
                             in_=xt_d[15, :, :]).then_inc(xmsem, 16)
            kgrp_dma(scalar, 6)
            xdma(scalar, 2, 0)
            xdma(scalar, 2, 1)
            kgrp_dma(scalar, 7)
            xdma(scalar, 2, 2)
            xdma(scalar, 2, 3)
            kgrp_dma(scalar, 8)
            xdma(scalar, 2, 4)
            kgrp_dma(scalar, 9)
            for j in range(NEX):
                scalar.dma_start(out=xex[j][:, :],
                                 in_=xe_d[j, :, :]).then_inc(xesems[j], 16)
            kgrp_dma(scalar, 10)
            scalar.wait_ge(psem, 9)
            scalar.copy(otile[:, 2 * NCH:3 * NCH], psum[2][:, :]).then_inc(vsemB, 1)
            scalar.wait_ge(psem, 10)
            scalar.copy(otile[:, 3 * NCH:], psum[3][:, :]).then_inc(vsemB, 1)


        @block.tensor
        def _(tensor):
            tensor.wait_ge(wsem, 1)
            for w in range(WARMUP):
                tensor.matmul(psum[0][:8, :64], wtile[:, :8], wtile[:, :],
                              start=True, stop=True)
            ci = 0
            kwaited = 0
            xtwaited = set()

            def kwait(ci):
                nonlocal kwaited
                need = kgrp_of(ci) + 1
                while kwaited < need:
                    tensor.wait_ge(kgsems[kwaited], 16)
                    kwaited += 1

            for s in range(NSLOT):
                for (xi, dz, sing) in _SLOT_SCHEDS[s]:
                    if xi == 5:
                        if 'mab' not in xtwaited:
                            tensor.wait_ge(xmsem, 16)
                            xtwaited.add('mab')
                    elif (s, xi) not in xtwaited:
                        tensor.wait_ge(xtsems[s][xi], 16)
                        xtwaited.add((s, xi))
                    kwait(ci)
                    src_tile = xmab if xi == 5 else xtiles[s][xi]
                    for m in range(NM):
                        tensor.matmul(
                            psum[m][:, :],
                            src_tile[:, dz + 2 * m:dz + 2 * m + 2, :, :],
                            ktile[:, ci * NCH:(ci + 1) * NCH],
                            start=(ci == 0), stop=False)
                    ci += 1
            for j in range(NEX):
                tensor.wait_ge(xesems[j], 16)
                kwait(ci)
                for m in range(NM):
                    mm = tensor.matmul(
                        psum[m][:, :],
                        xex[j][:, m * 128:(m + 1) * 128],
                        ktile[:, ci * NCH:(ci + 1) * NCH],
                        start=False, stop=(j == NEX - 1))
                    if j == NEX - 1:
                        mm.then_inc(psem, NM - m)
                ci += 1

        @block.vector
        def _(vector):
            vector.memset(wtile[:, :], 0.0).then_inc(wsem, 1)
            vector.wait_ge(psem, 4)
            vector.tensor_copy(otile[:, :NCH], psum[0][:, :]).then_inc(vsemA, 1)
            vector.wait_ge(psem, 7)
            vector.tensor_copy(otile[:, NCH:2 * NCH], psum[1][:, :]).then_inc(vsemA, 1)

        @block.gpsimd
        def _(gpsimd):
            gpsimd.wait_ge(vsemB, 2)
            gpsimd.dma_start(out=out_d[:, 2 * NCH:],
                             in_=otile[:, 2 * NCH:]).then_inc(osem, 16)
            gpsimd.wait_ge(osem, 32)

    _PROGRAM = nc
    return nc


def kernel(x, q_in, q_out, w_ss, w_vs, w_sv, w_vv0, w_vv1, bias):
    global LAST
    from concourse.bass_utils import run_bass_kernel_spmd

    kern = _assemble_kern(np.asarray(q_in, np.float32), np.asarray(q_out, np.float32),
                          np.asarray(w_ss, np.float32), np.asarray(w_vs, np.float32),
                          np.asarray(w_sv, np.float32), np.asarray(w_vv0, np.float32),
                          np.asarray(w_vv1, np.float32))
    xr = np.asarray(x, np.float32).reshape(NCH, P, P, P)
    x_pad = np.zeros((NCH, P + 4, P + 4, P + 4), np.float16)
    x_pad[:, 2:10, 2:10, 2:10] = xr.astype(np.float16)
    kerT = np.ascontiguousarray(kern.transpose(2, 1, 0)).astype(np.float16)  # (tap, C, A)

    in_maps = []
    for c in range(NCORES):
        cols = [c, 8 + c, 16 + c]
        xt = np.zeros((NSLOT * 5 + 1, 128, 768), np.float16)
        kchunks = np.zeros((NCHUNK, 128, NCH), np.float16)
        wins = []
        for s, col in enumerate(cols):
            dy, dx = col // 5, col % 5
            w = x_pad[:, :, dy:dy + 8, dx:dx + 8]          # (320, 12, 8, 8)
            wins.append(w)
            xt[5 * s + 0] = w[0:128].reshape(128, 768)
            xt[5 * s + 1] = w[128:256].reshape(128, 768)
            x3 = np.zeros((128, 12, 8, 8), np.float16)
            x3[:64] = w[256:320]
            x3[64:, :11] = w[0:64, 1:12]
            xt[5 * s + 2] = x3.reshape(128, 768)
            xt[5 * s + 3] = w[64:192].reshape(128, 768)
            xt[5 * s + 4] = w[192:320].reshape(128, 768)
        xm = np.zeros((128, 12, 8, 8), np.float16)
        xm[:64] = wins[0][256:320]
        xm[64:] = wins[1][0:64]
        xt[15] = xm.reshape(128, 768)

        ci = 0
        for s, col in enumerate(cols):
            def tap(d, col=col):
                return 25 * d + col
            for (xi, dz, sing) in _SLOT_SCHEDS[s]:
                if xi == 0:
                    kchunks[ci] = kerT[tap(dz), 0:128]
                elif xi == 1:
                    kchunks[ci] = kerT[tap(dz), 128:256]
                elif xi == 2 and sing:
                    kchunks[ci, :64] = kerT[tap(4), 256:320]
                elif xi == 2:
                    kchunks[ci, :64] = kerT[tap(dz), 256:320]
                    kchunks[ci, 64:] = kerT[tap(dz + 1), 0:64]
                elif xi == 3:
                    kchunks[ci] = kerT[tap(dz), 64:192]
                elif xi == 4:
                    kchunks[ci] = kerT[tap(dz), 192:320]
                elif xi == 5:
                    kchunks[ci, :64] = kerT[25 * 4 + cols[0], 256:320]
                    kchunks[ci, 64:] = kerT[25 * 4 + cols[1], 0:64]
                ci += 1
        # column 24: rows stream (5 taps x 320 ch) -> 16 chunks, 2 per core
        col = 24
        dy, dx = col // 5, col % 5
        w24 = x_pad[:, :, dy:dy + 8, dx:dx + 8]
        xrows = np.zeros((5 * NCH, NV), np.float16)
        krows = np.zeros((5 * NCH, NCH), np.float16)
        for dz in range(5):
            t = 25 * dz + col
            xrows[dz * NCH:(dz + 1) * NCH] = \
                w24[:, dz:dz + 8].reshape(NCH, NV)
            krows[dz * NCH:(dz + 1) * NCH] = kerT[t]
        xe = np.zeros((NEX, 128, NV), np.float16)
        for j in range(NEX):
            r0 = (2 * c + j) * 128
            r1 = min(r0 + 128, 5 * NCH)
            if r0 < 5 * NCH:
                xe[j, :r1 - r0] = xrows[r0:r1]
                kchunks[ci, :r1 - r0] = krows[r0:r1]
            ci += 1
        assert ci == NCHUNK, ci
        ker_c = np.ascontiguousarray(
            kchunks.transpose(1, 0, 2).reshape(128, NCHUNK * NCH))
        in_maps.append({"ker": ker_c, "xt": xt, "xe": xe})

    nc = _build_program()
    res = run_bass_kernel_spmd(nc, in_maps, list(range(NCORES)))
    LAST = res

    out = np.zeros((NV, NCH), np.float32)
    for c in range(NCORES):
        out += res.results[c]["out_part"].reshape(128, NM, NCH)\
            .transpose(1, 0, 2).reshape(NV, NCH).astype(np.float32)
    out = np.ascontiguousarray(out.T).reshape(1, DIM, Q, P, P, P)
    out[:, :C0] += np.asarray(bias, np.float32).reshape(1, C0, 1, 1, 1, 1)
    return out


# revision 6
# speedup vs baseline: 1.0189x; 1.0028x over previous
"""Equivariant PQ-layer conv kernel for 8x TRN2 NeuronCores — v4.

Sharding: 125 taps = 25 (dy,dx) columns x 5 dz. Cores own 3 columns each
(columns 0..23); column 24 is split across cores as 2 row-chunks each.
Within a column, the 5 taps differ only by a z-shift, so x ships as 5
per-core (dy,dx)-translated window tiles (128ch, 12z, 8y, 8x) that the PE
slices with program-constant 3D APs — no im2col duplication. Tap pairs
(dz,dz+1) chain the 320-row contraction into full 128-row K-chunks; the
dz=4 singles chain across the core's 3 columns via a cross-column mixed
tile, giving 40 K-chunks total. Transposed matmuls: psum[m] (128 vox,
320 A) accumulates all chunks at full PE clock (warmup matmuls ride out
the p-state ramp during the initial DMA); out partials are fp16, summed
on host. All semaphore waits are exact full counts on dedicated sems —
DMA completions can reorder across the 16 engines, so k-of-n threshold
waits on a shared sem are racy.
"""
import numpy as np

C0, C1 = 8, 4
K = 5
G = 8
EPS = 1e-6
R_MAX = 5.5
DIM = C0 + 3 * C1          # 20
Q = 16
P = 8
NCH = DIM * Q              # 320
NV = P * P * P             # 512
K3 = K ** 5 // K ** 2      # 125
NCORES = 8
NSLOT = 3                  # columns per core
NEX = 2                    # extra chunks (column 24 split)
NCHUNK = 40                # 12 + 13 + 13 + 2
NM = 4
WARMUP = 56

LAST = None
_PROGRAM = None


def _levi_civita():
    e = np.zeros((3, 3, 3), np.float32)
    e[0, 1, 2] = e[1, 2, 0] = e[2, 0, 1] = 1.0
    e[0, 2, 1] = e[2, 1, 0] = e[1, 0, 2] = -1.0
    return e


def _assemble_kern(q_in, q_out, w_ss, w_vs, w_sv, w_vv0, w_vv1):
    offs = np.arange(K, dtype=np.float32) - (K - 1) / 2.0
    oz, oy, ox = np.meshgrid(offs, offs, offs, indexing='ij')
    p_off = np.stack([oz, oy, ox], -1).reshape(-1, 3)
    v = p_off[None, None] - (q_out[:, None, None] - q_in[None, :, None])
    r = np.linalg.norm(v, axis=-1)
    u = np.where(r[..., None] > EPS, v / np.maximum(r, EPS)[..., None], 0.0).astype(np.float32)
    centers = np.linspace(0.0, R_MAX, G).astype(np.float32)
    sigma = R_MAX / (G - 1)
    R = np.exp(-0.5 * ((r[..., None] - centers) / sigma) ** 2).astype(np.float32)
    RY = R[..., None] * u[..., None, :]
    eye3 = np.eye(3, dtype=np.float32)
    eps3 = _levi_civita()
    K_ss = np.einsum('acg,pqkg->apcqk', w_ss, R, optimize=True)
    K_vs = np.einsum('acg,pqkgm->ampcqk', w_vs, RY, optimize=True)
    K_sv = np.einsum('acg,pqkgm->apcmqk', w_sv, RY, optimize=True)
    K_vv = (np.einsum('acg,pqkg,mn->ampcnqk', w_vv0, R, eye3, optimize=True)
            + np.float32(0.7071067811865476) *
            np.einsum('acg,pqkgm,imj->aipcjqk', w_vv1, RY, eps3, optimize=True))
    Qo, Qi = q_out.shape[0], q_in.shape[0]
    top = np.concatenate([K_ss, K_sv.reshape(C0, Qo, 3 * C1, Qi, K3)], axis=2)
    bot = np.concatenate([K_vs.reshape(3 * C1, Qo, C0, Qi, K3),
                          K_vv.reshape(3 * C1, Qo, 3 * C1, Qi, K3)], axis=2)
    kern = np.concatenate([top, bot], axis=0)
    return np.ascontiguousarray(kern.reshape(DIM * Qo, DIM * Qi, K3).astype(np.float32))


# chunk schedule within one column: (x-tile index 0..4, dz window offset)
# tiles: 0=X1 ch0:128, 1=X2 ch128:256, 2=X3 mixed (ch256:320 | ch0:64 @z+1),
#        3=X4 ch64:192, 4=X5 ch192:320
def _mk_slot_scheds():
    pairs = []
    for dz0 in (0, 2):
        pairs += [(0, dz0, False), (1, dz0, False), (2, dz0, False),
                  (3, dz0 + 1, False), (4, dz0 + 1, False)]
    s0 = [pairs[0], (0, 4, True), pairs[1], (1, 4, True), pairs[2],
          pairs[5], pairs[3], pairs[4]] + pairs[6:]
    s1 = pairs + [(3, 4, True), (4, 4, True), (5, 4, True)]
    s2 = pairs + [(0, 4, True), (1, 4, True), (2, 4, True)]
    return [s0, s1, s2]

_SLOT_SCHEDS = _mk_slot_scheds()
assert sum(len(s) for s in _SLOT_SCHEDS) + NEX == NCHUNK



def _zext(s, i):
    """(z_extent, z_base) of x-tile i in slot s: trimmed tiles drop unread
    z-slices. tiles 3,4 of slots 0,2: dz in {1,3} -> z 1..10; tile 2 of
    slots 0,1: dz in {0,2} -> z 0..9."""
    if i in (3, 4) and s in (0, 2):
        return 10, 1
    if i == 2 and s in (0, 1):
        return 10, 0
    return 12, 0

def _build_program():
    global _PROGRAM
    if _PROGRAM is not None:
        return _PROGRAM
    from contextlib import ExitStack
    from concourse import bass, mybir

    nc = bass.Bass("TRN2", target_bir_lowering=False, debug=False,
                   enable_asserts=False, num_devices=NCORES)
    ker_d = nc.dram_tensor("ker", [128, NCHUNK * NCH], mybir.dt.float16,
                           kind="ExternalInput").ap()
    xt_d = nc.dram_tensor("xt", [NSLOT * 5 + 1, 128, 768], mybir.dt.float16,
                          kind="ExternalInput").ap()
    xe_d = nc.dram_tensor("xe", [NEX, 128, NV], mybir.dt.float16,
                          kind="ExternalInput").ap()
    out_d = nc.dram_tensor("out_part", [128, NM * NCH], mybir.dt.float16,
                           kind="ExternalOutput").ap()

    # ker DMA groups: fine-grained so PE chunk waits are never far ahead
    KGRP = [(0, 1), (1, 2), (3, 4), (7, 5), (12, 5), (17, 5), (22, 3),
            (25, 5), (30, 5), (35, 3), (38, 2)]
    def kgrp_of(ci):
        for g, (c0, n) in enumerate(KGRP):
            if ci < c0 + n:
                return g
        raise AssertionError(ci)

    # 1-based positions of each resource in the scalar DMA stream;
    # chunk ci is runnable once dsem >= 16 * NEED[ci].
    kpos = {}
    xpos = {}
    _order = [('kg', 0), ('x', 0, 0), ('kg', 1), ('x', 0, 1), ('x', 0, 2),
          ('kg', 2), ('x', 0, 3), ('x', 0, 4), ('kg', 3), ('x', 1, 0),
          ('x', 1, 1), ('kg', 4), ('x', 1, 2), ('x', 1, 3), ('kg', 5),
          ('x', 1, 4), ('xmab',), ('kg', 6), ('x', 2, 0), ('x', 2, 1),
          ('kg', 7), ('x', 2, 2), ('x', 2, 3), ('kg', 8), ('x', 2, 4),
          ('kg', 9), ('xe', 0), ('xe', 1), ('kg', 10)]
    for pos, ent in enumerate(_order, 1):
        if ent[0] == 'kg':
            c0, n = KGRP[ent[1]]
            for ch in range(c0, c0 + n):
                kpos[ch] = pos
        elif ent[0] == 'x':
            xpos[(ent[1], ent[2])] = pos
        elif ent[0] == 'xmab':
            xpos['mab'] = pos
        else:
            xpos[('e', ent[1])] = pos


    with ExitStack() as ctx:
        ktile = ctx.enter_context(nc.sbuf_tensor("kt", [128, NCHUNK * NCH],
                                                 mybir.dt.float16))
        xtiles = [[ctx.enter_context(nc.sbuf_tensor(f"x{s}_{i}",
                                                    [128, _zext(s, i)[0], 8, 8],
                                                    mybir.dt.float16))
                   for i in range(5)] for s in range(NSLOT)]
        xmab = ctx.enter_context(nc.sbuf_tensor("xmab", [128, 12, 8, 8],
                                                mybir.dt.float16))
        xex = [ctx.enter_context(nc.sbuf_tensor(f"xe{j}", [128, NV], mybir.dt.float16))
               for j in range(NEX)]
        otile = ctx.enter_context(nc.sbuf_tensor("otile", [128, NM * NCH],
                                                 mybir.dt.float16))
        wtile = ctx.enter_context(nc.sbuf_tensor("wtile", [128, 64], mybir.dt.float16))
        psum = [ctx.enter_context(nc.psum_tensor(f"psum{m}", [128, NCH], mybir.dt.float32))
                for m in range(NM)]
        kgsems = [ctx.enter_context(nc.semaphore(f"kg{g}")) for g in range(len(KGRP))]
        xtsems = [[ctx.enter_context(nc.semaphore(f"xts{s}_{i}")) for i in range(5)]
                  for s in range(NSLOT)]
        xmsem = ctx.enter_context(nc.semaphore("xmsem"))
        xesems = [ctx.enter_context(nc.semaphore(f"xes{j}")) for j in range(NEX)]
        wsem = ctx.enter_context(nc.semaphore("wsem"))
        psem = ctx.enter_context(nc.semaphore("psem"))
        vsemA = ctx.enter_context(nc.semaphore("vsemA"))
        vsemB = ctx.enter_context(nc.semaphore("vsemB"))
        osem = ctx.enter_context(nc.semaphore("osem"))
        block = ctx.enter_context(nc.Block())

        def kgrp_dma(q, g):
            c0, nch_ = KGRP[g]
            q.dma_start(out=ktile[:, c0 * NCH:(c0 + nch_) * NCH],
                        in_=ker_d[:, c0 * NCH:(c0 + nch_) * NCH]
                        ).then_inc(kgsems[g], 16)

        @block.sync
        def _(sync):
            sync.wait_ge(vsemA, 2)
            sync.dma_start(out=out_d[:, :2 * NCH],
                           in_=otile[:, :2 * NCH]).then_inc(osem, 16)

        @block.scalar
        def _(scalar):
            # single hand-ordered stream: ker chunk groups interleave x tiles
            # exactly in consumption order, so the PE never starves.
            def xdma(scalar, s, i):
                ze = _zext(s, i)[0]
                scalar.dma_start(out=xtiles[s][i][:, :, :, :],
                                 in_=xt_d[5 * s + i, :, :ze * 64]
                                 ).then_inc(xtsems[s][i], 16)
            kgrp_dma(scalar, 0)
            xdma(scalar, 0, 0)
            kgrp_dma(scalar, 1)
            xdma(scalar, 0, 1)
            xdma(scalar, 0, 2)
            kgrp_dma(scalar, 2)
            xdma(scalar, 0, 3)
            xdma(scalar, 0, 4)
            kgrp_dma(scalar, 3)
            xdma(scalar, 1, 0)
            xdma(scalar, 1, 1)
            kgrp_dma(scalar, 4)
            xdma(scalar, 1, 2)
            xdma(scalar, 1, 3)
            kgrp_dma(scalar, 5)
            xdma(scalar, 1, 4)
            scalar.dma_start(out=xmab[:, :, :, :],
                             in_=xt_d[15, :, :]).then_inc(xmsem, 16)
            kgrp_dma(scalar, 6)
            xdma(scalar, 2, 0)
            xdma(scalar, 2, 1)
            kgrp_dma(scalar, 7)
            xdma(scalar, 2, 2)
            xdma(scalar, 2, 3)
            kgrp_dma(scalar, 8)
            xdma(scalar, 2, 4)
            kgrp_dma(scalar, 9)
            for j in range(NEX):
                scalar.dma_start(out=xex[j][:, :],
                                 in_=xe_d[j, :, :]).then_inc(xesems[j], 16)
            kgrp_dma(scalar, 10)
            scalar.wait_ge(psem, 9)
            scalar.copy(otile[:, 2 * NCH:3 * NCH], psum[2][:, :]).then_inc(vsemB, 1)
            scalar.wait_ge(psem, 10)
            scalar.copy(otile[:, 3 * NCH:], psum[3][:, :]).then_inc(vsemB, 1)


        @block.tensor
        def _(tensor):
            tensor.wait_ge(wsem, 1)
            for w in range(WARMUP):
                tensor.matmul(psum[0][:8, :64], wtile[:, :8], wtile[:, :],
                              start=True, stop=True)
            ci = 0
            kwaited = 0
            xtwaited = set()

            def kwait(ci):
                nonlocal kwaited
                need = kgrp_of(ci) + 1
                while kwaited < need:
                    tensor.wait_ge(kgsems[kwaited], 16)
                    kwaited += 1

            for s in range(NSLOT):
                for (xi, dz, sing) in _SLOT_SCHEDS[s]:
                    if xi == 5:
                        if 'mab' not in xtwaited:
                            tensor.wait_ge(xmsem, 16)
                            xtwaited.add('mab')
                    elif (s, xi) not in xtwaited:
                        tensor.wait_ge(xtsems[s][xi], 16)
                        xtwaited.add((s, xi))
                    kwait(ci)
                    if xi == 5:
                        src_tile, zb = xmab, 0
                    else:
                        src_tile, zb = xtiles[s][xi], _zext(s, xi)[1]
                    for m in range(NM):
                        z0 = dz - zb + 2 * m
                        tensor.matmul(
                            psum[m][:, :],
                            src_tile[:, z0:z0 + 2, :, :],
                            ktile[:, ci * NCH:(ci + 1) * NCH],
                            start=(ci == 0), stop=False)
                    ci += 1
            for j in range(NEX):
                tensor.wait_ge(xesems[j], 16)
                kwait(ci)
                for m in range(NM):
                    mm = tensor.matmul(
                        psum[m][:, :],
                        xex[j][:, m * 128:(m + 1) * 128],
                        ktile[:, ci * NCH:(ci + 1) * NCH],
                        start=False, stop=(j == NEX - 1))
                    if j == NEX - 1:
                        mm.then_inc(psem, NM - m)
                ci += 1

        @block.vector
        def _(vector):
            vector.memset(wtile[:, :], 0.0).then_inc(wsem, 1)
            vector.wait_ge(psem, 4)
            vector.tensor_copy(otile[:, :NCH], psum[0][:, :]).then_inc(vsemA, 1)
            vector.wait_ge(psem, 7)
            vector.tensor_copy(otile[:, NCH:2 * NCH], psum[1][:, :]).then_inc(vsemA, 1)

        @block.gpsimd
        def _(gpsimd):
            gpsimd.wait_ge(vsemB, 2)
            gpsimd.dma_start(out=out_d[:, 2 * NCH:],
                             in_=otile[:, 2 * NCH:]).then_inc(osem, 16)
            gpsimd.wait_ge(osem, 32)

    _PROGRAM = nc
    return nc


def kernel(x, q_in, q_out, w_ss, w_vs, w_sv, w_vv0, w_vv1, bias):
    global LAST
    from concourse.bass_utils import run_bass_kernel_spmd

    kern = _assemble_kern(np.asarray(q_in, np.float32), np.asarray(q_out, np.float32),
                          np.asarray(w_ss, np.float32), np.asarray(w_vs, np.float32),
                          np.asarray(w_sv, np.float32), np.asarray(w_vv0, np.float32),
                          np.asarray(w_vv1, np.float32))
    xr = np.asarray(x, np.float32).reshape(NCH, P, P, P)
    x_pad = np.zeros((NCH, P + 4, P + 4, P + 4), np.float16)
    x_pad[:, 2:10, 2:10, 2:10] = xr.astype(np.float16)
    kerT = np.ascontiguousarray(kern.transpose(2, 1, 0)).astype(np.float16)  # (tap, C, A)

    in_maps = []
    for c in range(NCORES):
        cols = [c, 8 + c, 16 + c]
        xt = np.zeros((NSLOT * 5 + 1, 128, 768), np.float16)
        kchunks = np.zeros((NCHUNK, 128, NCH), np.float16)
        wins = []
        for s, col in enumerate(cols):
            dy, dx = col // 5, col % 5
            w = x_pad[:, :, dy:dy + 8, dx:dx + 8]          # (320, 12, 8, 8)
            wins.append(w)
            xt[5 * s + 0] = w[0:128].reshape(128, 768)
            xt[5 * s + 1] = w[128:256].reshape(128, 768)
            ze2 = _zext(s, 2)[0]
            x3 = np.zeros((128, ze2, 8, 8), np.float16)
            x3[:64] = w[256:320, 0:ze2]
            x3[64:, :min(ze2, 11)] = w[0:64, 1:1 + min(ze2, 11)]
            xt[5 * s + 2, :, :ze2 * 64] = x3.reshape(128, ze2 * 64)
            ze4, zb4 = _zext(s, 3)
            xt[5 * s + 3, :, :ze4 * 64] = \
                w[64:192, zb4:zb4 + ze4].reshape(128, ze4 * 64)
            xt[5 * s + 4, :, :ze4 * 64] = \
                w[192:320, zb4:zb4 + ze4].reshape(128, ze4 * 64)
        xm = np.zeros((128, 12, 8, 8), np.float16)
        xm[:64] = wins[0][256:320]
        xm[64:] = wins[1][0:64]
        xt[15] = xm.reshape(128, 768)

        ci = 0
        for s, col in enumerate(cols):
            def tap(d, col=col):
                return 25 * d + col
            for (xi, dz, sing) in _SLOT_SCHEDS[s]:
                if xi == 0:
                    kchunks[ci] = kerT[tap(dz), 0:128]
                elif xi == 1:
                    kchunks[ci] = kerT[tap(dz), 128:256]
                elif xi == 2 and sing:
                    kchunks[ci, :64] = kerT[tap(4), 256:320]
                elif xi == 2:
                    kchunks[ci, :64] = kerT[tap(dz), 256:320]
                    kchunks[ci, 64:] = kerT[tap(dz + 1), 0:64]
                elif xi == 3:
                    kchunks[ci] = kerT[tap(dz), 64:192]
                elif xi == 4:
                    kchunks[ci] = kerT[tap(dz), 192:320]
                elif xi == 5:
                    kchunks[ci, :64] = kerT[25 * 4 + cols[0], 256:320]
                    kchunks[ci, 64:] = kerT[25 * 4 + cols[1], 0:64]
                ci += 1
        # column 24: rows stream (5 taps x 320 ch) -> 16 chunks, 2 per core
        col = 24
        dy, dx = col // 5, col % 5
        w24 = x_pad[:, :, dy:dy + 8, dx:dx + 8]
        xrows = np.zeros((5 * NCH, NV), np.float16)
        krows = np.zeros((5 * NCH, NCH), np.float16)
        for dz in range(5):
            t = 25 * dz + col
            xrows[dz * NCH:(dz + 1) * NCH] = \
                w24[:, dz:dz + 8].reshape(NCH, NV)
            krows[dz * NCH:(dz + 1) * NCH] = kerT[t]
        xe = np.zeros((NEX, 128, NV), np.float16)
        for j in range(NEX):
            r0 = (2 * c + j) * 128
            r1 = min(r0 + 128, 5 * NCH)
            if r0 < 5 * NCH:
                xe[j, :r1 - r0] = xrows[r0:r1]
                kchunks[ci, :r1 - r0] = krows[r0:r1]
            ci += 1
        assert ci == NCHUNK, ci
        ker_c = np.ascontiguousarray(
            kchunks.transpose(1, 0, 2).reshape(128, NCHUNK * NCH))
        in_maps.append({"ker": ker_c, "xt": xt, "xe": xe})

    nc = _build_program()
    res = run_bass_kernel_spmd(nc, in_maps, list(range(NCORES)))
    LAST = res

    out = np.zeros((NV, NCH), np.float32)
    for c in range(NCORES):
        out += res.results[c]["out_part"].reshape(128, NM, NCH)\
            .transpose(1, 0, 2).reshape(NV, NCH).astype(np.float32)
    out = np.ascontiguousarray(out.T).reshape(1, DIM, Q, P, P, P)
    out[:, :C0] += np.asarray(bias, np.float32).reshape(1, C0, 1, 1, 1, 1)
    return out
